# revision 1
# baseline (speedup 1.0000x reference)
"""Trainium2 Bass kernel for EnhancedTransformerBlock on ragged graphs.

Layout: transposed activations [channels (partitions), nodes (free)].
Sharding: 64 graphs -> 8 cores x 8 slots, assigned by size-sorted rank so
slot widths (uniform across cores, required for SPMD) hug the max count.
All per-graph segment ops become free-dim slices; GraphNorm stats come from
bn_stats over the zero-padded slot slice plus closed-form corrections.
"""

import math
import numpy as np
import ml_dtypes

N_CORES = 8
B = 64
H = 256
NH = 8
HD = H // NH
EPS = 1e-5
F32 = None  # set on import of mybir below

import concourse.bass as bass
import concourse.bacc as bacc
import concourse.mybir as mybir
import concourse.tile as tile
from concourse.bass_utils import run_bass_kernel_spmd
from contextlib import ExitStack

F32 = mybir.dt.float32
F32R = mybir.dt.float32r
BF16 = mybir.dt.bfloat16
AF = mybir.ActivationFunctionType
OP = mybir.AluOpType

NEG = -1.0e9       # additive key mask (pre-exp)
BIG = 1.0e30       # sumexp offset at padded query columns


def _plan(batch):
    batch = np.asarray(batch).astype(np.int64)
    counts = np.bincount(batch, minlength=B).astype(np.int64)
    starts = np.concatenate([[0], np.cumsum(counts)[:-1]])
    order = np.argsort(-counts, kind="stable")  # rank -> graph id
    NS = B // N_CORES  # slots per core
    Ms, slot_graph = [], np.zeros((N_CORES, NS), np.int64)
    for s in range(NS):
        blk = order[N_CORES * s: N_CORES * s + N_CORES]
        m = int(max(16, math.ceil(max(1, counts[blk].max()) / 16) * 16))
        Ms.append(m)
        for c in range(N_CORES):
            slot_graph[c, s] = blk[c]
    offs = np.concatenate([[0], np.cumsum(Ms)]).astype(np.int64)
    Rtot = int(offs[-1])
    R = int(math.ceil(Rtot / 128) * 128)
    return counts, starts, slot_graph, Ms, offs, Rtot, R


def _qchunks(m):
    # query-range chunks <=256 (PSUM bank budget for the 8-head score tile)
    out = []
    o = 0
    while o < m:
        c = min(256, m - o)
        out.append((o, c))
        o += c
    return out


def _build(nc, Ms, offs, R):
    NS = len(Ms)
    nkt = [math.ceil(m / 128) for m in Ms]
    NKT = sum(nkt)
    SC = 1.0 / math.sqrt(HD)

    # ---- DRAM tensors ----
    d_xt = nc.dram_tensor("xt", [2, 128, R], F32, kind="ExternalInput").ap()
    d_wqk = nc.dram_tensor("wqk", [2, 128, 512], F32R, kind="ExternalInput").ap()
    d_wv = nc.dram_tensor("wv", [2, 128, 256], F32R, kind="ExternalInput").ap()
    d_wo = nc.dram_tensor("wo", [2, 128, 256], F32R, kind="ExternalInput").ap()
    d_w1 = nc.dram_tensor("w1", [2, 128, 1024], F32R, kind="ExternalInput").ap()
    d_w2 = nc.dram_tensor("w2", [8, 128, 256], F32R, kind="ExternalInput").ap()
    # packed per-partition constants, column layout:
    # [qkb(4) ob(2) fb1(8) fb2(2) n1w(2) n1b(2) n2w(2) n2b(2) ga1(NS) gA(NS) gB(NS) km(NKT) ident(128)]
    NCST = 24 + 3 * NS + NKT
    d_cst = nc.dram_tensor("cst", [128, NCST], F32, kind="ExternalInput").ap()
    d_qm = nc.dram_tensor("qm", [1, R], BF16, kind="ExternalInput").ap()        # 0/BIG at padded q
    d_onesb = nc.dram_tensor("onesb", [128, 128], BF16, kind="ExternalInput").ap()
    d_zz = nc.dram_tensor("zz", [128, 2048], F32R, kind="ExternalInput").ap()
    d_ot = nc.dram_tensor("ot", [2, 128, R], F32, kind="ExternalOutput").ap()

    def mm(out, lhsT, rhs, **kw):
        nc.tensor.matmul(out, lhsT, rhs, **kw)

    with tile.TileContext(nc) as tc, ExitStack() as ctx:
        pers = ctx.enter_context(tc.tile_pool(name="pers", bufs=1))
        ptp = ctx.enter_context(tc.tile_pool(name="ptp", bufs=12))
        hgp = ctx.enter_context(tc.tile_pool(name="hgp", bufs=3))
        stat = ctx.enter_context(tc.tile_pool(name="stat", bufs=4))
        psA = ctx.enter_context(tc.tile_pool(name="psA", bufs=4, space="PSUM"))

        # ---- load inputs ----
        def load(name, dram, shape, dt=F32):
            t = pers.tile(shape, dt, tag=name)
            nc.sync.dma_start(out=t, in_=dram)
            return t

        xt = []
        for i in range(2):
            t = pers.tile([128, R], F32, name=f"xt{i}", tag=f"xt{i}")
            nc.sync.dma_start(out=t[:, :R // 2], in_=d_xt[i][:, :R // 2])
            nc.sync.dma_start(out=t[:, R // 2:], in_=d_xt[i][:, R // 2:])
            xt.append(t)
        wqk = [load(f"wqk{i}", d_wqk[i], [128, 512], F32R) for i in range(2)]
        wv = [load(f"wv{i}", d_wv[i], [128, 256], F32R) for i in range(2)]
        wo = [load(f"wo{i}", d_wo[i], [128, 256], F32R) for i in range(2)]
        w1 = [load(f"w1{i}", d_w1[i], [128, 1024], F32R) for i in range(2)]
        w2 = [load(f"w2{i}", d_w2[i], [128, 256], F32R) for i in range(8)]
        cst = load("cst", d_cst, [128, NCST])
        co = 0
        def cslice(n):
            nonlocal co
            a = cst[:, co:co + n]; co += n
            return a
        qkb = [cslice(1) for _ in range(4)]
        ob = [cslice(1) for _ in range(2)]
        fb1 = [cslice(1) for _ in range(8)]
        fb2 = [cslice(1) for _ in range(2)]
        nw = [[cslice(1) for _ in range(2)] for _ in range(2)]
        nb = [[cslice(1) for _ in range(2)] for _ in range(2)]
        ga1 = cslice(NS)
        gA = cslice(NS)
        gB = cslice(NS)
        km = [cslice(1) for _ in range(NKT)]
        qm = load("qm", d_qm, [1, R], BF16)
        onesb = load("onesb", d_onesb, [128, 128], BF16)
        ones1 = onesb[0:1, :]
        oneskt = onesb[:, 0:32]
        qZall = load("qZall", d_zz, [128, 2048], F32R)
        qZ = [qZall[:, 256 * h:256 * h + 256] for h in range(8)]

        NCH = [(o, min(512, R - o)) for o in range(0, R, 512)]

        # ---------- GraphNorm (shared) ----------
        def gnorm(src, dst, widx):
            # per (chtile, slot-half) stats via bn_stats over the padded slice,
            # then corrections for the zero padding (unbiased var, eps on std)
            NH2 = NS // 2
            for ct in range(2):
                for h2 in range(2):
                    sl = range(h2 * NH2, (h2 + 1) * NH2)
                    c0 = h2 * NH2
                    mv = stat.tile([128, 2, NH2], F32, name="mv", tag=f"mv{ct}{h2}")
                    for s in sl:
                        st6 = stat.tile([128, 6], F32, name="st6", tag="st6")
                        nc.vector.bn_stats(out=st6, in_=src[ct][:, offs[s]:offs[s] + Ms[s]])
                        nc.vector.bn_aggr(out=mv[:, :, s - c0:s - c0 + 1], in_=st6)
                    mean_r = mv[:, 0:1, :].squeeze(1)
                    var_r = mv[:, 1:2, :].squeeze(1)
                    m2 = stat.tile([128, NH2], F32, name="m2", tag="m2")
                    nc.vector.tensor_mul(m2, mean_r, mean_r)
                    v1 = stat.tile([128, NH2], F32, name="v1", tag="v1")
                    nc.vector.tensor_mul(v1, var_r, gA[:, c0:c0 + NH2])
                    v2 = stat.tile([128, NH2], F32, name="v2", tag="v2")
                    nc.vector.tensor_mul(v2, m2, gB[:, c0:c0 + NH2])
                    var = stat.tile([128, NH2], F32, name="var", tag="var")
                    nc.vector.tensor_add(var, v1, v2)
                    # std = exp(0.5*ln(var)) + EPS (stays in the exp/ln ACT set)
                    lnv = stat.tile([128, NH2], F32, name="lnv", tag="lnv")
                    nc.scalar.activation(out=lnv, in_=var, func=AF.Ln)
                    std = stat.tile([128, NH2], F32, name="std", tag="std")
                    nc.scalar.activation(out=std, in_=lnv, func=AF.Exp, scale=0.5)
                    nc.vector.tensor_scalar_add(std, std, EPS)
                    rstd = stat.tile([128, NH2], F32, name="rstd", tag="rstd")
                    scr = stat.tile([128, NH2], F32, name="scr", tag="scr")
                    nc.vector.reciprocal_approx_accurate(out=rstd, in_=std, scratch=scr)
                    mean = stat.tile([128, NH2], F32, name="mean", tag="mean")
                    nc.vector.tensor_mul(mean, mean_r, ga1[:, c0:c0 + NH2])
                    scale = stat.tile([128, NH2], F32, name="scale", tag="scale")
                    nc.vector.tensor_scalar_mul(scale, rstd, nw[widx][ct])
                    shift = stat.tile([128, NH2], F32, name="shift", tag="shift")
                    nc.vector.tensor_mul(shift, mean, scale)
                    nc.vector.tensor_scalar(
                        out=shift, in0=shift, scalar1=-1.0, scalar2=nb[widx][ct],
                        op0=OP.mult, op1=OP.add,
                    )
                    for s in sl:
                        nc.vector.tensor_scalar(
                            out=dst[ct][:, offs[s]:offs[s] + Ms[s]],
                            in0=src[ct][:, offs[s]:offs[s] + Ms[s]],
                            scalar1=scale[:, s - c0:s - c0 + 1],
                            scalar2=shift[:, s - c0:s - c0 + 1],
                            op0=OP.mult, op1=OP.add,
                        )

        # ---------- phase 1: gnorm1 ----------
        xn = [pers.tile([128, R], F32R, name=f"xn{i}", tag=f"xn{i}") for i in range(2)]
        Rtot = offs[-1]
        if R > Rtot:
            for ct in range(2):
                nc.sync.dma_start(out=xn[ct][:, Rtot:R], in_=d_zz[:, :R - Rtot])
        gnorm(xt, xn, 0)

        # ---------- phase 2: q,k  (qk[mt] = rows 128*mt of [q;k] = W_qk @ xn) ----
        qk = [pers.tile([128, R], F32R, name=f"qk{m}", tag=f"qk{m}") for m in range(4)]
        for mt in range(4):
            for (o, w) in NCH:
                ps = psA.tile([128, 512], F32, name="ps1", tag="ps1")
                for kt in range(2):
                    mm(ps[:, :w], wqk[kt][:, 128 * mt:128 * mt + 128],
                       xn[kt][:, o:o + w], start=(kt == 0), stop=(kt == 1))
                nc.scalar.activation(out=qk[mt][:, o:o + w], in_=ps[:, :w],
                                     func=AF.Identity, bias=qkb[mt])
        # ---------- phase 2b: vRows per (slot, ktile)  [keys, 256] ----------
        vr = pers.tile([128, 256 * NKT], BF16, name="vr", tag="vr")
        vri = {}
        idx = 0
        for s in range(NS):
            for kt in range(nkt[s]):
                vri[(s, kt)] = idx
                mkt = min(128, Ms[s] - 128 * kt)
                ko = offs[s] + 128 * kt
                ps = psA.tile([128, 512], F32, name="ps1", tag="ps1")
                for ct in range(2):
                    mm(ps[:mkt, :256], xn[ct][:, ko:ko + mkt], wv[ct],
                       start=(ct == 0), stop=(ct == 1))
                nc.vector.tensor_copy(vr[:mkt, 256 * idx:256 * idx + 256], ps[:mkt, :256])
                idx += 1

        # ---------- phase 3: attention per (slot, qchunk) ----------
        ctxt = [pers.tile([128, R], F32R, name=f"ctx{i}", tag=f"ctx{i}") for i in range(2)]
        if R > Rtot:
            for ct in range(2):
                nc.sync.dma_start(out=ctxt[ct][:, Rtot:R], in_=d_zz[:, :R - Rtot])
        kmi = {}
        idx = 0
        for s in range(NS):
            for kt in range(nkt[s]):
                kmi[(s, kt)] = idx
                idx += 1
        with tc.tile_pool(name="psST", bufs=2, space="PSUM") as psST:
            for s in range(NS):
                for (qo, qc) in _qchunks(Ms[s]):
                    qbase = offs[s] + qo
                    for h in range(8):
                        hp = 32 * (h % 4)
                        nc.vector.tensor_copy(
                            qZ[h][hp:hp + 32, :qc],
                            qk[h // 4][hp:hp + 32, qbase:qbase + qc])
                    pts = []
                    for kt in range(nkt[s]):
                        mkt = min(128, Ms[s] - 128 * kt)
                        ko = offs[s] + 128 * kt
                        ph = []
                        for g in range(2):
                            st = psST.tile([128, 4 * 256], F32, name="st", tag="st")
                            for j in range(4):
                                h = 4 * g + j
                                lhsT = qk[2 + h // 4][:, ko:ko + mkt]
                                mm(st[:mkt, j * qc:(j + 1) * qc], lhsT, qZ[h][:, :qc],
                                   start=True, stop=True)
                            pt = ptp.tile([128, 4 * 256], BF16, name="pt", tag="pt")
                            nc.scalar.activation(
                                out=pt[:mkt, :4 * qc], in_=st[:mkt, :4 * qc],
                                func=AF.Exp, bias=km[kmi[(s, kt)]][:mkt], scale=SC)
                            ph.append(pt)
                        pts.append(ph)
                    cs = [psA.tile([128, 512], F32, name="cs", tag="ps1") for _ in range(2)]
                    for g in range(2):
                        mm(cs[g][:, qc:2 * qc], ones1[:, :128], qm[:, qbase:qbase + qc],
                           start=True, stop=False)
                    for kt in range(nkt[s]):
                        mkt = min(128, Ms[s] - 128 * kt)
                        vb = 256 * vri[(s, kt)]
                        last = kt == nkt[s] - 1
                        for g in range(2):
                            for j in range(4):
                                h = 4 * g + j
                                mm(cs[g][32 * j:32 * j + 32, 0:qc],
                                   vr[:mkt, vb + 32 * h:vb + 32 * h + 32],
                                   pts[kt][g][:mkt, j * qc:(j + 1) * qc],
                                   start=(kt == 0), stop=last, tile_position=(0, 32 * j))
                                mm(cs[g][32 * j:32 * j + 32, qc:2 * qc],
                                   oneskt[:mkt, :],
                                   pts[kt][g][:mkt, j * qc:(j + 1) * qc],
                                   start=False, stop=last, tile_position=(0, 32 * j))
                    for g in range(2):
                        rec = stat.tile([128, 256], F32, name="rec", tag="rec")
                        nc.vector.reciprocal_approx_fast(out=rec[:, :qc], in_=cs[g][:, qc:2 * qc])
                        nc.vector.tensor_mul(
                            ctxt[g][:, qbase:qbase + qc], cs[g][:, 0:qc], rec[:, :qc])

        # ---------- phase 4: out_proj + residual -> x2 ----------
        x2 = [pers.tile([128, R], F32, name=f"x2{i}", tag=f"x2{i}") for i in range(2)]
        for ct in range(2):
            for (o, w) in NCH:
                ps = psA.tile([128, 512], F32, name="ps1", tag="ps1")
                for kt in range(2):
                    mm(ps[:, :w], wo[kt][:, 128 * ct:128 * ct + 128],
                       ctxt[kt][:, o:o + w], start=(kt == 0), stop=(kt == 1))
                nc.vector.scalar_tensor_tensor(
                    out=x2[ct][:, o:o + w], in0=ps[:, :w], scalar=ob[ct],
                    in1=xt[ct][:, o:o + w], op0=OP.add, op1=OP.add,
                )

        # ---------- phase 5: gnorm2 ----------
        # reuse xn slots; dead zone is still zero from phase 1
        xn2 = [pers.tile([128, R], F32R, name=f"xn{i}", tag=f"xn{i}") for i in range(2)]
        gnorm(x2, xn2, 1)

        # ---------- phase 6: FFN ----------
        out_t = [pers.tile([128, R], F32, name=f"xt{i}", tag=f"xt{i}") for i in range(2)]  # reuse xt slots
        half = int(offs[NS // 2])
        HCH = []
        for lo, hi in ((0, half), (half, R)):
            o = lo
            while o < hi:
                w = min(1024, hi - o)
                HCH.append((o, w))
                o += w
        with tc.tile_pool(name="psH", bufs=2, space="PSUM") as psH:
            for (o, w) in HCH:
                hg = []
                for mt in range(8):
                    ps = psH.tile([128, 1024], F32, name="hps", tag="hps")
                    for o2 in range(0, w, 512):
                        wc = min(512, w - o2)
                        for kt in range(2):
                            mm(ps[:, o2:o2 + wc], w1[kt][:, 128 * mt:128 * mt + 128],
                               xn2[kt][:, o + o2:o + o2 + wc],
                               start=(kt == 0), stop=(kt == 1))
                    h = hgp.tile([128, 1024], F32R, name="hg", tag="hg")
                    nc.scalar.activation(out=h[:, :w], in_=ps[:, :w],
                                         func=AF.Gelu, bias=fb1[mt])
                    hg.append(h)
                for ct in range(2):
                    for o2 in range(0, w, 512):
                        w2c = min(512, w - o2)
                        ps2 = psA.tile([128, 512], F32, name="ps2", tag="ps1")
                        for kt in range(8):
                            mm(ps2[:, :w2c], w2[kt][:, 128 * ct:128 * ct + 128],
                               hg[kt][:, o2:o2 + w2c], start=(kt == 0), stop=(kt == 7))
                        nc.vector.scalar_tensor_tensor(
                            out=out_t[ct][:, o + o2:o + o2 + w2c], in0=ps2[:, :w2c],
                            scalar=fb2[ct], in1=x2[ct][:, o + o2:o + o2 + w2c],
                            op0=OP.add, op1=OP.add)
                        nc.sync.dma_start(out=d_ot[ct][:, o + o2:o + o2 + w2c],
                                          in_=out_t[ct][:, o + o2:o + o2 + w2c])
    return nc


_CACHE = {}


def _prepare(inputs):
    x = np.asarray(inputs["x"], np.float32)
    batch = np.asarray(inputs["batch"]).astype(np.int64)
    counts, starts, slot_graph, Ms, offs, Rtot, R = _plan(batch)
    NS = len(Ms)
    nkt = [math.ceil(m / 128) for m in Ms]
    NKT = sum(nkt)

    in_proj_w = np.asarray(inputs["in_proj_w"], np.float32)
    in_proj_b = np.asarray(inputs["in_proj_b"], np.float32)
    out_proj_w = np.asarray(inputs["out_proj_w"], np.float32)
    out_proj_b = np.asarray(inputs["out_proj_b"], np.float32)
    ffn_w1 = np.asarray(inputs["ffn_w1"], np.float32)
    ffn_b1 = np.asarray(inputs["ffn_b1"], np.float32)
    ffn_w2 = np.asarray(inputs["ffn_w2"], np.float32)
    ffn_b2 = np.asarray(inputs["ffn_b2"], np.float32)

    # fold the v-branch input bias through out_proj (exact, linear)
    ob_eff = out_proj_b + out_proj_w @ in_proj_b[2 * H:3 * H]

    wqk = np.ascontiguousarray(in_proj_w[:2 * H].T.reshape(2, 128, 512))
    wv = np.ascontiguousarray(in_proj_w[2 * H:].T.reshape(2, 128, 256))
    wo = np.ascontiguousarray(out_proj_w.T.reshape(2, 128, 256))
    w1 = np.ascontiguousarray(ffn_w1.T.reshape(2, 128, 1024))
    w2 = np.ascontiguousarray(ffn_w2.T.reshape(8, 128, 256))
    qkb = np.ascontiguousarray(in_proj_b[:2 * H].reshape(4, 128, 1))
    ob = np.ascontiguousarray(ob_eff.reshape(2, 128, 1))
    fb1 = np.ascontiguousarray(ffn_b1.reshape(8, 128, 1))
    fb2 = np.ascontiguousarray(ffn_b2.reshape(2, 128, 1))
    nw = np.stack([np.asarray(inputs["norm1_w"], np.float32).reshape(2, 128, 1),
                   np.asarray(inputs["norm2_w"], np.float32).reshape(2, 128, 1)])
    nb = np.stack([np.asarray(inputs["norm1_b"], np.float32).reshape(2, 128, 1),
                   np.asarray(inputs["norm2_b"], np.float32).reshape(2, 128, 1)])

    xT = x.T  # [256, N]
    xts = np.zeros((N_CORES, 2, 128, R), np.float32)
    ga1 = np.zeros((N_CORES, 128, NS), np.float32)
    gA = np.zeros((N_CORES, 128, NS), np.float32)
    gB = np.zeros((N_CORES, 128, NS), np.float32)
    kms = np.full((N_CORES, NKT, 128, 1), NEG, np.float32)
    qms = np.zeros((N_CORES, 1, R), np.float32)
    onesb = np.ones((128, 128), ml_dtypes.bfloat16)
    zz = np.zeros((128, 2048), np.float32)
    for c in range(N_CORES):
        for s in range(NS):
            g = slot_graph[c, s]
            n = int(counts[g])
            st = int(starts[g])
            o = int(offs[s])
            if n > 0:
                blk = xT[:, st:st + n]
                xts[c, 0, :, o:o + n] = blk[:128]
                xts[c, 1, :, o:o + n] = blk[128:]
            ne = max(n, 1)
            ga1[c, :, s] = Ms[s] / ne
            inv_nm1 = 1.0 / max(ne - 1, 1)
            gA[c, :, s] = Ms[s] * inv_nm1
            gB[c, :, s] = Ms[s] * (1.0 - Ms[s] / ne) * inv_nm1
            ki = sum(nkt[:s])
            for kt in range(nkt[s]):
                v = min(128, max(0, n - 128 * kt))
                kms[c, ki + kt, :v, 0] = 0.0
            qms[c, 0, o + n:o + Ms[s]] = BIG
        qms[c, 0, Rtot:R] = BIG

    key = (tuple(Ms), R)
    if key not in _CACHE:
        nc = bacc.Bacc("TRN2", target_bir_lowering=False, debug=False,
                       num_devices=N_CORES)
        _build(nc, Ms, offs, R)
        nc.compile()
        _CACHE[key] = nc
    nc = _CACHE[key]

    in_maps = []
    for c in range(N_CORES):
        in_maps.append({
            "xt": xts[c], "wqk": wqk, "wv": wv, "wo": wo, "w1": w1, "w2": w2,
            "cst": np.ascontiguousarray(np.concatenate(
                [qkb[:, :, 0].T, ob[:, :, 0].T, fb1[:, :, 0].T, fb2[:, :, 0].T,
                 nw.reshape(4, 128).T, nb.reshape(4, 128).T,
                 ga1[c], gA[c], gB[c],
                 kms[c][:, :, 0].T], axis=1).astype(np.float32)),
            "qm": qms[c].astype(ml_dtypes.bfloat16),
            "onesb": onesb, "zz": zz,
        })

    def unpack(outs):
        out = np.empty((x.shape[0], H), np.float32)
        for c in range(N_CORES):
            ot = outs[c]["ot"]  # [2, 128, R]
            full = np.concatenate([ot[0], ot[1]], axis=0)  # [256, R]
            for s in range(NS):
                g = slot_graph[c, s]
                n = int(counts[g])
                st = int(starts[g])
                o = int(offs[s])
                if n > 0:
                    out[st:st + n] = full[:, o:o + n].T
        return out

    return nc, in_maps, unpack


def kernel(**inputs):
    nc, in_maps, unpack = _prepare(inputs)
    res = run_bass_kernel_spmd(nc, in_maps, list(range(N_CORES)))
    return unpack(res.results)


def _traced_run(**inputs):
    """Cost-model timeline (single core) + warm wall-clock. Returns model ns."""
    import time
    nc, in_maps, unpack = _prepare(inputs)
    t0 = time.time()
    run_bass_kernel_spmd(nc, in_maps, list(range(N_CORES)))
    t1 = time.time()
    run_bass_kernel_spmd(nc, in_maps, list(range(N_CORES)))
    t2 = time.time()
    print(f"wall cold: {t1 - t0:.2f}s  warm: {t2 - t1:.2f}s")
    from concourse.timeline_sim import TimelineSim
    import trails.perfetto as _tp
    for _m in ("enable_explicit_ordering", "reserve_process_order",
               "reserve_thread_order", "set_process_order", "set_thread_order",
               "add_instant"):
        if not hasattr(_tp.LazyPerfetto, _m):
            setattr(_tp.LazyPerfetto, _m, lambda self, *a, **k: None)
    if not hasattr(_tp.LazyPerfetto, "add_counter"):
        def _add_counter(self, *a, **k):
            try:
                self.update_counter(*a, **k)
            except Exception:
                pass
        _tp.LazyPerfetto.add_counter = _add_counter
    tl = TimelineSim(nc, trace=True)
    total = tl.simulate()
    pf = tl.perfetto
    if callable(pf):
        pf = pf()
    if pf is not None:
        try:
            pf.save("/root/problem/tl.perfetto-trace")
        except Exception as e:
            print("perfetto dump failed:", e)
    return total



# revision 8
# speedup vs baseline: 1.0819x; 1.0819x over previous
"""Trainium2 Bass kernel for EnhancedTransformerBlock on ragged graphs.

Layout: transposed activations [channels (partitions), nodes (free)].
Sharding: 64 graphs -> 8 cores x 8 slots, assigned by size-sorted rank so
slot widths (uniform across cores, required for SPMD) hug the max count.

v2 design notes (vs the phase-batched f32r baseline):
- all matmul activations bf16; FFN + out_proj in fp8e4m3 with DoubleRow
  (contraction 256 per matmul at 0.5 cyc/row).
- scores: zero-padded per-head q replica (qZ) built once per slot with 4x-mode
  DVE copies; contraction 128 (4 heads of k x zero-trick).
- PV transposed: out [q<=128, 33] per head; col 33h+32 of vr holds 0.125 so the
  same matmul accumulates sumexp/8 (fp8 range prep for ctx).
- divide = stride-0 broadcast tensor_tensor; back-transpose on PE (identity).
- k needs no bias (cancels in softmax over keys); out_proj bias + wo@v_bias +
  ffn_b2 pre-added to x on host (GraphNorm is per-channel shift invariant).
- per-slot pipelining: attention(s) -> out_proj(s) -> gnorm2 stats(s); FFN per
  half interleaved with the other half's attention.
"""

import math
import numpy as np
import ml_dtypes

import concourse.bass as bass
import concourse.bacc as bacc
import concourse.mybir as mybir
import concourse.tile as tile
from concourse.bass_utils import run_bass_kernel_spmd
from contextlib import ExitStack

N_CORES = 8
B = 64
H = 256
NH = 8
HD = H // NH
EPS = 1e-5

F32 = mybir.dt.float32
BF16 = mybir.dt.bfloat16
FP8 = mybir.dt.float8e4
AF = mybir.ActivationFunctionType
OP = mybir.AluOpType
PM = mybir.MatmulPerfMode

NEG = -30.0          # additive key mask (pre-exp); exp(-30) == 0 in bf16
SC = 1.0 / math.sqrt(HD)
S1 = 32.0            # ffn_w1 fp8 prescale
S2 = 32.0            # ffn_w2 fp8 prescale
SO = 32.0            # out_proj_w fp8 prescale
SCX = 8.0            # ctx fp8 prescale (via 1/8 in the vr ones-column)


def _plan(batch):
    batch = np.asarray(batch).astype(np.int64)
    counts = np.bincount(batch, minlength=B).astype(np.int64)
    starts = np.concatenate([[0], np.cumsum(counts)[:-1]])
    order = np.argsort(-counts, kind="stable")  # rank -> graph id
    NS = B // N_CORES
    Ms, slot_graph = [], np.zeros((N_CORES, NS), np.int64)
    for s in range(NS):
        blk = order[N_CORES * s: N_CORES * s + N_CORES]
        m = int(max(16, math.ceil(max(1, counts[blk].max()) / 16) * 16))
        Ms.append(m)
        for c in range(N_CORES):
            slot_graph[c, s] = blk[c]
    offs = np.concatenate([[0], np.cumsum(Ms)]).astype(np.int64)
    Rtot = int(offs[-1])
    R = int(math.ceil(Rtot / 128) * 128)
    return counts, starts, slot_graph, Ms, offs, Rtot, R


def _build(nc, Ms, offs, R, ns_valid):
    """ns_valid[s] = max valid node count over cores for slot s (<= Ms[s]).
    Per-core valid counts differ; we compute the slot at the max width and the
    km mask (per core) zeroes the prob rows beyond each core's own count.
    Query-side trims use ns_valid (same extent every core keeps SPMD single
    program); pads beyond ns_valid are never read back by any core."""
    NS = len(Ms)
    nkt = [math.ceil(m / 128) for m in Ms]
    NKT = sum(nkt)
    MMAX = max(Ms)

    # ---- DRAM ----
    d_xt = nc.dram_tensor("xt", [2, 128, R], BF16, kind="ExternalInput").ap()
    d_wqk = nc.dram_tensor("wqk", [2, 128, 512], BF16, kind="ExternalInput").ap()
    d_wvo = nc.dram_tensor("wvo", [2, 128, 264], BF16, kind="ExternalInput").ap()
    d_wo = nc.dram_tensor("wo", [2, 128, 2, 128], FP8, kind="ExternalInput").ap()
    d_w1 = nc.dram_tensor("w1", [128, 2, 1024], FP8, kind="ExternalInput").ap()
    d_w2 = nc.dram_tensor("w2", [8, 128, 2, 128], FP8, kind="ExternalInput").ap()
    # packed per-partition constants:
    # [qb(2) fb1(8) n1w(2) n1b(2) n2w(2) n2b(2) ga1(NS) gA(NS) gB(NS) km(NKT)]
    NCST = 18 + 3 * NS + NKT
    d_cst = nc.dram_tensor("cst", [128, NCST], F32, kind="ExternalInput").ap()
    d_id = nc.dram_tensor("ident", [128, 128], BF16, kind="ExternalInput").ap()
    d_ot = nc.dram_tensor("ot", [2, 128, R], BF16, kind="ExternalOutput").ap()

    mm = nc.tensor.matmul

    with tile.TileContext(nc) as tc, ExitStack() as ctx:
        pers = ctx.enter_context(tc.tile_pool(name="pers", bufs=1))
        ptp = ctx.enter_context(tc.tile_pool(name="ptp", bufs=4))
        hgp = ctx.enter_context(tc.tile_pool(name="hgp", bufs=2))
        stat = ctx.enter_context(tc.tile_pool(name="stat", bufs=4))
        ctxp = ctx.enter_context(tc.tile_pool(name="ctxp", bufs=3))
        psP = ctx.enter_context(tc.tile_pool(name="psP", bufs=2, space="PSUM"))
        psS = ctx.enter_context(tc.tile_pool(name="psS", bufs=2, space="PSUM"))
        psC = ctx.enter_context(tc.tile_pool(name="psC", bufs=2, space="PSUM"))
        psT = ctx.enter_context(tc.tile_pool(name="psT", bufs=2, space="PSUM"))

        # ---- persistent SBUF tiles ----
        cst = pers.tile([128, NCST], F32, name="cst", tag="cst")
        nc.sync.dma_start(out=cst, in_=d_cst)
        co = 0
        def cslice(n):
            nonlocal co
            a = cst[:, co:co + n]; co += n
            return a
        qb = [cslice(1) for _ in range(2)]
        fb1 = [cslice(1) for _ in range(8)]
        nw = [[cslice(1) for _ in range(2)] for _ in range(2)]
        nb = [[cslice(1) for _ in range(2)] for _ in range(2)]
        ga1 = cslice(NS)
        gA = cslice(NS)
        gB = cslice(NS)
        km = [cslice(1) for _ in range(NKT)]
        kmi = {}
        idx = 0
        for s in range(NS):
            for kt in range(nkt[s]):
                kmi[(s, kt)] = idx; idx += 1

        ident = pers.tile([128, 128], BF16, name="ident", tag="ident")
        nc.sync.dma_start(out=ident, in_=d_id)

        xt = [pers.tile([128, R], BF16, name=f"xt{i}", tag=f"xt{i}") for i in range(2)]
        for ct in range(2):
            for s in range(NS):
                nc.sync.dma_start(out=xt[ct][:, offs[s]:offs[s] + Ms[s]],
                                  in_=d_xt[ct][:, offs[s]:offs[s] + Ms[s]])
        wqk = [pers.tile([128, 512], BF16, name=f"wqk{i}", tag=f"wqk{i}") for i in range(2)]
        wvo = [pers.tile([128, 264], BF16, name=f"wvo{i}", tag=f"wvo{i}") for i in range(2)]
        for i in range(2):
            nc.sync.dma_start(out=wqk[i], in_=d_wqk[i])
            nc.sync.dma_start(out=wvo[i], in_=d_wvo[i])
        wo = [pers.tile([128, 2, 128], FP8, name=f"wo{i}", tag=f"wo{i}") for i in range(2)]
        for i in range(2):
            nc.sync.dma_start(out=wo[i], in_=d_wo[i])
        w1 = pers.tile([128, 2, 1024], FP8, name="w1", tag="w1")
        nc.sync.dma_start(out=w1, in_=d_w1)
        w2 = [pers.tile([128, 2, 128], FP8, name=f"w2{i}", tag=f"w2{i}") for i in range(8)]
        for i in range(8):
            nc.sync.dma_start(out=w2[i], in_=d_w2[i])

        xn = [pers.tile([128, R], BF16, name=f"xn{i}", tag=f"xn{i}") for i in range(2)]
        qt_ = [pers.tile([128, R], BF16, name=f"q{i}", tag=f"q{i}") for i in range(2)]
        kt_ = [pers.tile([128, R], BF16, name=f"k{i}", tag=f"k{i}") for i in range(2)]
        qZ = pers.tile([128, 8, MMAX], BF16, name="qZ", tag="qZ")
        nc.gpsimd.memset(qZ, 0.0)  # persistent zeros; head h only ever writes rows 32*(h%4)
        vr = pers.tile([128, 264 * NKT], BF16, name="vr", tag="vr")
        ctxt = pers.tile([128, 2, R], FP8, name="ctxt", tag="ctxt")
        x2 = [pers.tile([128, R], BF16, name=f"x2{i}", tag=f"x2{i}") for i in range(2)]
        for ct in range(2):
            nc.gpsimd.memset(x2[ct], 0.0)  # pads must stay 0 for gnorm2 stats
        xn2 = pers.tile([128, 2, R], FP8, name="xn2", tag="xn2")
        out_t = [pers.tile([128, R], BF16, name=f"ot{i}", tag=f"ot{i}") for i in range(2)]

        # ---------- GraphNorm stats+apply ----------
        def gnorm(src_f, dst_f, widx, slots, awidths):
            # src_f(ct) -> AP [128, R]; dst_f(ct, s) -> out AP for slot slice
            for ct in range(2):
                nsl = len(slots)
                mv = stat.tile([128, 2, nsl], F32, name="mv", tag=f"mv{ct}")
                for i, s in enumerate(slots):
                    st6 = stat.tile([128, 6], F32, name="st6", tag="st6")
                    nc.vector.bn_stats(out=st6, in_=src_f(ct)[:, offs[s]:offs[s] + Ms[s]])
                    nc.vector.bn_aggr(out=mv[:, :, i:i + 1], in_=st6)
                c0 = slots[0]
                mean_r = mv[:, 0:1, :].squeeze(1)
                var_r = mv[:, 1:2, :].squeeze(1)
                m2 = stat.tile([128, nsl], F32, name="m2", tag="m2")
                nc.vector.tensor_mul(m2, mean_r, mean_r)
                v1 = stat.tile([128, nsl], F32, name="v1", tag="v1")
                nc.vector.tensor_mul(v1, var_r, gA[:, c0:c0 + nsl])
                var = stat.tile([128, nsl], F32, name="var", tag="var")
                nc.vector.tensor_mul(var, m2, gB[:, c0:c0 + nsl])
                nc.vector.tensor_add(var, var, v1)
                lnv = stat.tile([128, nsl], F32, name="lnv", tag="lnv")
                nc.scalar.activation(out=lnv, in_=var, func=AF.Ln)
                std = stat.tile([128, nsl], F32, name="std", tag="std")
                nc.scalar.activation(out=std, in_=lnv, func=AF.Exp, scale=0.5)
                nc.vector.tensor_scalar_add(std, std, EPS)
                rstd = stat.tile([128, nsl], F32, name="rstd", tag="rstd")
                scr = stat.tile([128, nsl], F32, name="scr", tag="scr")
                nc.vector.reciprocal_approx_accurate(out=rstd, in_=std, scratch=scr)
                mean = stat.tile([128, nsl], F32, name="mean", tag="mean")
                nc.vector.tensor_mul(mean, mean_r, ga1[:, c0:c0 + nsl])
                scale = stat.tile([128, nsl], F32, name="scale", tag="scale")
                nc.vector.tensor_scalar_mul(scale, rstd, nw[widx][ct])
                shift = stat.tile([128, nsl], F32, name="shift", tag="shift")
                nc.vector.tensor_mul(shift, mean, scale)
                nc.vector.tensor_scalar(
                    out=shift, in0=shift, scalar1=-1.0, scalar2=nb[widx][ct],
                    op0=OP.mult, op1=OP.add)
                for i, s in enumerate(slots):
                    w = awidths[s]
                    nc.vector.tensor_scalar(
                        out=dst_f(ct, s, w),
                        in0=src_f(ct)[:, offs[s]:offs[s] + w],
                        scalar1=scale[:, i:i + 1],
                        scalar2=shift[:, i:i + 1],
                        op0=OP.mult, op1=OP.add)

        # ---------- phase 1: gnorm1 (apply over full Ms: pads finite) ----------
        HALVES = [list(range(0, NS // 2)), list(range(NS // 2, NS))]
        for half in HALVES:
            gnorm(lambda ct: xt[ct],
                  lambda ct, s, w: xn[ct][:, offs[s]:offs[s] + w],
                  0, half, {s: Ms[s] for s in range(NS)})

        # ---------- phase 2: q,k projections ----------
        NCH = [(o, min(512, R - o)) for o in range(0, R, 512)]
        for mt in range(4):
            for (o, w) in NCH:
                ps = psP.tile([128, 512], F32, name="psp", tag="psp")
                for ktc in range(2):
                    mm(ps[:, :w], wqk[ktc][:, 128 * mt:128 * mt + 128],
                       xn[ktc][:, o:o + w], start=(ktc == 0), stop=(ktc == 1))
                if mt < 2:  # q: add bias (k bias cancels in softmax)
                    nc.vector.tensor_scalar_add(qt_[mt][:, o:o + w], ps[:, :w], qb[mt])
                else:
                    nc.vector.tensor_copy(kt_[mt - 2][:, o:o + w], ps[:, :w])

        # ---------- phase 2b: v rows per (slot, ktile), cols 33h..33h+32 ----
        vri = {}
        idx = 0
        for s in range(NS):
            for kt in range(nkt[s]):
                vri[(s, kt)] = idx; idx += 1
        def vproj(s, kt):
            mkt = min(128, Ms[s] - 128 * kt)
            ko = offs[s] + 128 * kt
            vb = 264 * vri[(s, kt)]
            ps = psP.tile([128, 512], F32, name="psp", tag="psp")
            for ctc in range(2):
                mm(ps[:mkt, :264], xn[ctc][:, ko:ko + mkt], wvo[ctc],
                   start=(ctc == 0), stop=(ctc == 1))
            nc.vector.tensor_copy(vr[:mkt, vb:vb + 264], ps[:mkt, :264])
            # sumexp ones-column = 1/SCX (ctx fp8 prescale rides the ratio)
            ones = vr[:mkt, vb:vb + 264].rearrange("p (h c) -> p h c", h=8, c=33)[:, :, 32:33].squeeze(2)
            nc.gpsimd.memset(ones, 1.0 / SCX)

        for s in range(NS):
            for kt in range(nkt[s]):
                vproj(s, kt)

        # ---------- attention per slot ----------
        def attn_slot(s):
            M = Ms[s]; nv = ns_valid[s]
            # qZ: per head, copy q rows 32j into the persistent zero tile
            for h in range(8):
                hp = 32 * (h % 4)
                nc.vector.tensor_copy(qZ[hp:hp + 32, h, :M],
                                      qt_[h // 4][hp:hp + 32, offs[s]:offs[s] + M])
            nq = math.ceil(nv / 128)
            for qi in range(nq):
                qo = 128 * qi
                qc = min(128, nv - qo)
                qbase = offs[s] + qo
                cs = psC.tile([128, 264], F32, name="cs", tag="cs")
                pts = []
                for kt in range(nkt[s]):
                    mkt = min(128, M - 128 * kt)
                    ko = offs[s] + 128 * kt
                    ph = []
                    for g in range(2):
                        st = psS.tile([128, 512], F32, name="st", tag="st")
                        for j in range(4):
                            h = 4 * g + j
                            mm(st[:mkt, 128 * j:128 * j + qc],
                               kt_[g][:, ko:ko + mkt], qZ[:, h, qo:qo + qc],
                               start=True, stop=True)
                        pt = ptp.tile([128, 512], BF16, name="pt", tag="pt")
                        stv = st[:mkt, :].rearrange("p (j c) -> p j c", j=4, c=128)[:, :, :qc]
                        ptv = pt[:mkt, :].rearrange("p (j c) -> p j c", j=4, c=128)[:, :, :qc]
                        nc.scalar.activation(out=ptv, in_=stv, func=AF.Exp,
                                             bias=km[kmi[(s, kt)]][:mkt], scale=SC)
                        ph.append(pt)
                    pts.append(ph)
                for kt in range(nkt[s]):
                    mkt = min(128, M - 128 * kt)
                    vb = 264 * vri[(s, kt)]
                    last = kt == nkt[s] - 1
                    for g in range(2):
                        for j in range(4):
                            h = 4 * g + j
                            mm(cs[:qc, 33 * h:33 * h + 33],
                               pts[kt][g][:mkt, 128 * j:128 * j + qc],
                               vr[:mkt, vb + 33 * h:vb + 33 * h + 33],
                               start=(kt == 0), stop=last)
                # rec = SCX / sumexp ; ctxT = cs * rec (broadcast over 33-blocks)
                rec = stat.tile([128, 8], F32, name="rec", tag="rec")
                den = cs[:qc, :].rearrange("p (h c) -> p h c", h=8, c=33)[:, :, 32:33].squeeze(2)
                nc.vector.reciprocal_approx_fast(out=rec[:qc, :], in_=den)
                ctxTs = ctxp.tile([128, 256], BF16, name="ctxTs", tag="ctxTs")
                csv = cs[:qc, :].rearrange("p (h c) -> p h c", h=8, c=33)[:, :, 0:32]
                ctv = ctxTs[:qc, :].rearrange("p (h c) -> p h c", h=8, c=32)
                rv = rec[:qc, :].unsqueeze(2).broadcast_to([qc, 8, 32])
                nc.vector.tensor_tensor(out=ctv, in0=csv, in1=rv, op=OP.mult)
                # transpose back: [qc, 256] -> ctxt[:, ct, qbase:qbase+qc]
                tp = psT.tile([128, 2, 128], BF16, name="tp", tag="tp")
                for ctc in range(2):
                    nc.tensor.transpose(tp[:, ctc, :qc], ctxTs[:qc, 128 * ctc:128 * ctc + 128],
                                        ident[:qc, :qc])
                nc.vector.tensor_copy(ctxt[:, :, qbase:qbase + qc], tp[:, :, :qc])

        # ---------- out_proj + residual (valid width only; pads stay 0) ----
        def outproj_slot(s):
            nv = ns_valid[s]
            o = offs[s]
            for ctc in range(2):
                ps = psP.tile([128, 512], F32, name="psp", tag="psp")
                for cw in range(0, nv, 256):  # DoubleRow rhs free = 2*w <= 512
                    w = min(256, nv - cw)
                    mm(ps[:, cw:cw + w], wo[ctc], ctxt[:, :, o + cw:o + cw + w],
                       start=True, stop=True, perf_mode=PM.DoubleRow)
                nc.vector.scalar_tensor_tensor(
                    out=x2[ctc][:, o:o + nv], in0=ps[:, :nv], scalar=1.0 / (SCX * SO),
                    in1=xt[ctc][:, o:o + nv], op0=OP.mult, op1=OP.add)

        # ---------- gnorm2 + FFN per slot ----------
        def ffn_slot(s):
            nv = ns_valid[s]
            o = offs[s]
            hg = [hgp.tile([128, 2, MMAX], FP8, name=f"hg{p}", tag=f"hg{p}") for p in range(4)]
            for mt in range(8):
                ps = psP.tile([128, 512], F32, name="psp", tag="psp")
                for cw in range(0, nv, 256):  # DoubleRow rhs free = 2*w <= 512
                    w = min(256, nv - cw)
                    mm(ps[:, cw:cw + w], w1[:, :, 128 * mt:128 * mt + 128],
                       xn2[:, :, o + cw:o + cw + w],
                       start=True, stop=True, perf_mode=PM.DoubleRow)
                nc.scalar.activation(out=hg[mt // 2][:, mt % 2, :nv], in_=ps[:, :nv],
                                     func=AF.Gelu, bias=fb1[mt], scale=1.0 / S1)
            for ctc in range(2):
                ps2 = psP.tile([128, 512], F32, name="psp", tag="psp")
                for cw in range(0, nv, 256):
                    w = min(256, nv - cw)
                    for p in range(4):
                        mm(ps2[:, cw:cw + w], w2[4 * ctc + p], hg[p][:, :, cw:cw + w],
                           start=(p == 0), stop=(p == 3), perf_mode=PM.DoubleRow)
                nc.vector.scalar_tensor_tensor(
                    out=out_t[ctc][:, o:o + nv], in0=ps2[:, :nv], scalar=1.0 / S2,
                    in1=x2[ctc][:, o:o + nv], op0=OP.mult, op1=OP.add)

        for hi, half in enumerate(HALVES):
            for s in half:
                attn_slot(s)
                outproj_slot(s)
            gnorm(lambda ct: x2[ct],
                  lambda ct, s, w: xn2[:, ct, offs[s]:offs[s] + w],
                  1, half, {s: ns_valid[s] for s in range(NS)})
            for s in half:
                ffn_slot(s)
            lo, hiw = offs[half[0]], offs[half[-1] + 1]
            for ctc in range(2):
                nc.sync.dma_start(out=d_ot[ctc][:, lo:hiw], in_=out_t[ctc][:, lo:hiw])
    return nc


_CACHE = {}


def _prepare(inputs):
    x = np.asarray(inputs["x"], np.float32)
    batch = np.asarray(inputs["batch"]).astype(np.int64)
    counts, starts, slot_graph, Ms, offs, Rtot, R = _plan(batch)
    NS = len(Ms)
    nkt = [math.ceil(m / 128) for m in Ms]
    NKT = sum(nkt)

    in_proj_w = np.asarray(inputs["in_proj_w"], np.float32)
    in_proj_b = np.asarray(inputs["in_proj_b"], np.float32)
    out_proj_w = np.asarray(inputs["out_proj_w"], np.float32)
    out_proj_b = np.asarray(inputs["out_proj_b"], np.float32)
    ffn_w1 = np.asarray(inputs["ffn_w1"], np.float32)
    ffn_b1 = np.asarray(inputs["ffn_b1"], np.float32)
    ffn_w2 = np.asarray(inputs["ffn_w2"], np.float32)
    ffn_b2 = np.asarray(inputs["ffn_b2"], np.float32)

    # biases folded into the residual stream (gnorm is shift-invariant):
    # x' = x + out_proj_b + wo @ v_bias + ffn_b2
    fold = out_proj_b + out_proj_w @ in_proj_b[2 * H:3 * H] + ffn_b2
    xb = x + fold[None, :]

    wqk = np.ascontiguousarray(in_proj_w[:2 * H].T.reshape(2, 128, 512)).astype(ml_dtypes.bfloat16)
    # wv expanded to 33-col stride with zero ones-columns
    wvT = in_proj_w[2 * H:].T.reshape(2, 128, 8, 32)
    wvo = np.zeros((2, 128, 8, 33), np.float32)
    wvo[:, :, :, :32] = wvT
    wvo = wvo.reshape(2, 128, 264).astype(ml_dtypes.bfloat16)
    # wo fp8 [ct_out][128, 2(plane=ct_in), 128], prescaled
    woT = (out_proj_w.T * SO).reshape(2, 128, 2, 128)   # [ct_in, part, ct_out, col]
    wo8 = np.ascontiguousarray(woT.transpose(2, 1, 0, 3)).astype(ml_dtypes.float8_e4m3)
    # w1 fp8 [128, 2, 1024]: plane = input ct
    w18 = np.ascontiguousarray((ffn_w1.T * S1).reshape(2, 128, 1024).transpose(1, 0, 2)).astype(ml_dtypes.float8_e4m3)
    # w2 fp8 [8][128, 2, 128]: idx = 4*ct_out + pair; plane i = hidden 256p+128i
    w2T = (ffn_w2.T * S2).reshape(4, 2, 128, 2, 128)    # [pair, plane, part, ct_out, col]
    w28 = np.ascontiguousarray(
        w2T.transpose(3, 0, 2, 1, 4).reshape(8, 128, 2, 128)).astype(ml_dtypes.float8_e4m3)

    qkb = in_proj_b[:2 * H].reshape(4, 128)
    fb1 = ffn_b1.reshape(8, 128)
    nwv = np.stack([np.asarray(inputs["norm1_w"], np.float32).reshape(2, 128),
                    np.asarray(inputs["norm2_w"], np.float32).reshape(2, 128)])
    nbv = np.stack([np.asarray(inputs["norm1_b"], np.float32).reshape(2, 128),
                    np.asarray(inputs["norm2_b"], np.float32).reshape(2, 128)])

    xT = xb.T  # [256, N]
    xts = np.zeros((N_CORES, 2, 128, R), np.float32)
    ga1 = np.zeros((N_CORES, 128, NS), np.float32)
    gA = np.zeros((N_CORES, 128, NS), np.float32)
    gB = np.zeros((N_CORES, 128, NS), np.float32)
    kms = np.full((N_CORES, NKT, 128), NEG, np.float32)
    ns_valid = [0] * NS
    for c in range(N_CORES):
        for s in range(NS):
            g = slot_graph[c, s]
            n = int(counts[g])
            st = int(starts[g])
            o = int(offs[s])
            ns_valid[s] = max(ns_valid[s], n)
            if n > 0:
                blk = xT[:, st:st + n]
                xts[c, 0, :, o:o + n] = blk[:128]
                xts[c, 1, :, o:o + n] = blk[128:]
            ne = max(n, 1)
            ga1[c, :, s] = Ms[s] / ne
            inv_nm1 = 1.0 / max(ne - 1, 1)
            gA[c, :, s] = Ms[s] * inv_nm1
            gB[c, :, s] = Ms[s] * (1.0 - Ms[s] / ne) * inv_nm1
            ki = sum(nkt[:s])
            for kt in range(nkt[s]):
                v = min(128, max(0, n - 128 * kt))
                kms[c, ki + kt, :v] = 0.0
    ns_valid = [int(math.ceil(v / 16) * 16) if v % 16 else v for v in ns_valid]
    ns_valid = [min(v, Ms[s]) for s, v in enumerate(ns_valid)]

    key = (tuple(Ms), R, tuple(ns_valid))
    if key not in _CACHE:
        nc = bacc.Bacc("TRN2", target_bir_lowering=False, debug=False,
                       num_devices=N_CORES)
        _build(nc, Ms, offs, R, ns_valid)
        nc.compile()
        _CACHE[key] = nc
    nc = _CACHE[key]

    ident = np.eye(128).astype(ml_dtypes.bfloat16)
    in_maps = []
    for c in range(N_CORES):
        cstc = np.concatenate(
            [qkb[:2].T,                       # qb (q bias only)
             fb1.T,                           # 8
             nwv.reshape(4, 128).T,           # 4
             nbv.reshape(4, 128).T,           # 4
             ga1[c], gA[c], gB[c],            # 3*NS
             kms[c].T], axis=1).astype(np.float32)
        in_maps.append({
            "xt": xts[c].astype(ml_dtypes.bfloat16),
            "wqk": wqk, "wvo": wvo, "wo": wo8, "w1": w18, "w2": w28,
            "cst": np.ascontiguousarray(cstc),
            "ident": ident,
        })

    def unpack(outs):
        out = np.empty((x.shape[0], H), np.float32)
        for c in range(N_CORES):
            ot = np.asarray(outs[c]["ot"]).astype(np.float32)  # [2, 128, R]
            full = np.concatenate([ot[0], ot[1]], axis=0)      # [256, R]
            for s in range(NS):
                g = slot_graph[c, s]
                n = int(counts[g])
                st = int(starts[g])
                o = int(offs[s])
                if n > 0:
                    out[st:st + n] = full[:, o:o + n].T
        return out

    return nc, in_maps, unpack


def kernel(**inputs):
    nc, in_maps, unpack = _prepare(inputs)
    res = run_bass_kernel_spmd(nc, in_maps, list(range(N_CORES)))
    return unpack(res.results)


def _traced_run(**inputs):
    """Cost-model timeline (single core) + warm wall-clock. Returns model ns."""
    import time
    nc, in_maps, unpack = _prepare(inputs)
    t0 = time.time()
    run_bass_kernel_spmd(nc, in_maps, list(range(N_CORES)))
    t1 = time.time()
    run_bass_kernel_spmd(nc, in_maps, list(range(N_CORES)))
    t2 = time.time()
    print(f"wall cold: {t1 - t0:.2f}s  warm: {t2 - t1:.2f}s")
    from concourse.timeline_sim import TimelineSim
    import trails.perfetto as _tp
    for _m in ("enable_explicit_ordering", "reserve_process_order",
               "reserve_thread_order", "set_process_order", "set_thread_order",
               "add_instant"):
        if not hasattr(_tp.LazyPerfetto, _m):
            setattr(_tp.LazyPerfetto, _m, lambda self, *a, **k: None)
    if not hasattr(_tp.LazyPerfetto, "add_counter"):
        def _add_counter(self, *a, **k):
            try:
                self.update_counter(*a, **k)
            except Exception:
                pass
        _tp.LazyPerfetto.add_counter = _add_counter
    tl = TimelineSim(nc, trace=True)
    total = tl.simulate()
    pf = tl.perfetto
    if callable(pf):
        pf = pf()
    if pf is not None:
        try:
            pf.save("/root/problem/tl.perfetto-trace")
        except Exception as e:
            print("perfetto dump failed:", e)
    return total
